# revision 10
# baseline (speedup 1.0000x reference)
"""Trainium2 Bass kernel for nn_NeptuneMoEModel (moe_routing).

Model: 6 small MLPs (router + 2 energy experts + 3 direction experts) over
N=262144 points -> segment-mean-pool into B=1024 events -> tiny per-event
head/mixing math.

v4 strategy (8 NeuronCores, SPMD, data-parallel over events):
  - Events sorted by point count, round-robin assigned to cores; slots
    first-fit packed into 2048-column windows (as v3 baseline).
  - L1 = fused [9 -> 1536] bf16 matmul; gelu on the scalar engine (ACT)
    writing h1 as fp8e4 into one combined [128, 12*WIN] tile.
  - L2 = fp8 DoubleRow matmuls (K=256 in one pass, 2 moving rows/cycle):
    W2 quantized to e4m3 at scale 16 plus an e4m3 residual-correction
    matmul, accumulating in fp32 PSUM.  ~1.7x faster than bf16 on the PE.
  - L2 gelu split between ACT (scale=1/16 + bias) and a custom DVE
    micro-op (8-ALU-stage clamped cubic sigmoid with exact tails; computes
    32*gelu(psum/16); host divides those pooled sums by 32).  This offloads
    the ACT bottleneck onto the otherwise underused vector engine.
  - Pooling: h2 lives in one combined [128, 12*WIN] tile; per slot one
    bf16 tensor_tensor halving pass + one [12, L/2] tensor_reduce
    (~3x fewer DVE cycles than v3's 12 separate reduces per slot).
  - Host: divide by counts, per-tile scales, head matmuls, softmax/gating
    mixing - all O(B*1536) numpy.

Falls back to an all-bf16/all-ACT program (v3 baseline behavior) when
biases are nonzero; setup_inputs uses zero biases so the harness always
takes the fast path.
"""

import sys

sys.path.insert(0, "/opt/trn_rl_repo")

import numpy as np

import concourse.bass as bass
import concourse.mybir as mybir
import concourse.tile as tile
from concourse import bacc

N_CORES = 8
B = 1024
N_PTS = 262144
DIN = 9
H = 256
NNETS = 6
ZDIMS = [6, 2, 2, 3, 3, 3]
ZOFF = [0, 6, 8, 10, 13, 16]
ZD = 19
WIN = 2048
PIECE = 512
SLOTS = B // N_CORES  # 128
F32 = mybir.dt.float32
BF16 = mybir.dt.bfloat16
FP8 = mybir.dt.float8e4
try:
    import ml_dtypes

    NPBF16 = ml_dtypes.bfloat16
    NPFP8 = ml_dtypes.float8_e4m3
except ImportError:  # pragma: no cover
    NPBF16 = None
    NPFP8 = None
GELU = mybir.ActivationFunctionType.Gelu_apprx_tanh

# ---- custom DVE gelu: out = x * (1 + u*(A2 + B2*u^2)), u = clamp(x, -GB, GB)
# == 2*gelu_approx(x); GB*(A2 + B2*GB^2) = 2*0.5 = 1 makes the tails exact
# (out = 2x for x > GB, out = 0 for x < -GB).
GELU_GB = 2.05995
_GA = 0.364086
_GBB = (0.5 / GELU_GB - _GA) / GELU_GB**2
GELU_A2 = 2 * _GA
GELU_B2 = 2 * _GBB

# which L2 j-tiles (j = 2*net + mo) run their gelu on the DVE custom op
DVE_L2 = (4, 5, 8, 9, 10, 11)
FP8_DR = True  # L2 via fp8 DoubleRow + residual
W2_SCALE = 16.0  # e4m3 quantization scale for W2 (and its residual)

_GELU_OP = None


def _register_gelu_op():
    """Register the custom DVE op at runtime (no repo edits needed)."""
    global _GELU_OP
    if _GELU_OP is not None:
        return _GELU_OP
    from concourse import dve_ops as dvo
    from concourse.dve_spec import Spec, Src0, C0, C1, C2, Zero, One, lower
    from concourse.dve_spec import maxx, minn, sq
    from concourse.dve_uop import DveOpSpec

    name = "GELU2X_CCS_ANT"
    for op in dvo.OPS:
        if op.name == name:
            _GELU_OP = op
            return op

    u = minn(maxx(Src0, C0), Zero - C0)
    body = ((sq(u) * C2 + C1) * u + One) * Src0

    def ref(in0, s0, s1, imm2):
        uu = np.clip(in0, s0, -s0)
        return ((uu * uu * imm2 + s1) * uu + 1.0) * in0

    spec = Spec(body=body, reference=ref)
    row = dvo._CUSTOM_DVE_ROW_BASE + len(dvo.OPS)
    assert row < 0x20, "custom DVE opcode rows exhausted"
    dvo._SUB_OPCODE_FOR_NAME[name] = row
    shas = {}
    for ver in ("v3", "v4"):
        o = DveOpSpec(name=name, opcode=row, uops=lower(spec, ver=ver), rd1_en=False)
        shas[ver] = o.sha(ver)
    op = dvo.DveOp(name=name, spec=spec, subdim=False, uops_sha=shas)
    dvo.OPS.append(op)
    dvo.CUSTOM_DVE_SPECS[name] = spec
    _GELU_OP = op
    return op


def _gelu(x):
    """jax.nn.gelu(approximate=True) in numpy/fp32."""
    x = np.asarray(x, np.float32)
    c = np.float32(np.sqrt(2.0 / np.pi))
    return (0.5 * x * (1.0 + np.tanh(c * (x + 0.044715 * x * x * x)))).astype(
        np.float32
    )


# ----------------------------------------------------------------------------
# Layout: event -> (core, slot); slots -> windows
# ----------------------------------------------------------------------------


def build_layout(counts):
    counts = np.asarray(counts)
    order = np.argsort(-counts, kind="stable")
    ev = order.reshape(SLOTS, N_CORES)  # ev[s, c] = event id
    slot_len = counts[ev].max(1)
    # multiples of 8 keep both halving passes' operand offsets even (4B
    # aligned) so the DVE 2x_1p packed mode stays engaged
    slot_len = np.maximum(((slot_len + 7) // 8) * 8, 8).astype(np.int64)
    assert slot_len.max() <= WIN
    # first-fit (slot_len is non-increasing -> this is first-fit-decreasing)
    win_used = []
    slot_win = np.zeros(SLOTS, np.int64)
    slot_off = np.zeros(SLOTS, np.int64)
    for s in range(SLOTS):
        L = int(slot_len[s])
        for w in range(len(win_used)):
            if win_used[w] + L <= WIN:
                slot_win[s] = w
                slot_off[s] = win_used[w]
                win_used[w] += L
                break
        else:
            slot_win[s] = len(win_used)
            slot_off[s] = 0
            win_used.append(L)
    nw = len(win_used)
    slots_per_win = [[] for _ in range(nw)]
    for s in range(SLOTS):
        slots_per_win[slot_win[s]].append(
            (s, int(slot_off[s]), int(slot_len[s]))
        )
    win_cols = [min(WIN, ((u + 7) // 8) * 8) for u in win_used]
    return dict(
        ev=ev,
        slot_len=slot_len,
        slot_win=slot_win,
        slot_off=slot_off,
        nw=nw,
        slots_per_win=slots_per_win,
        win_cols=win_cols,
    )


# ----------------------------------------------------------------------------
# Device program (v4)
# ----------------------------------------------------------------------------


def build_program(nw, slots_per_win, win_cols=None, slots=SLOTS, act=GELU,
                  dve_l2=DVE_L2, fp8_dr=FP8_DR):
    gelu_op = _register_gelu_op()
    nc = bacc.Bacc(None, target_bir_lowering=False)
    if win_cols is None:
        win_cols = [WIN] * nw
    S = nw * WIN
    h1_dt = FP8 if fp8_dr else BF16
    xin = nc.dram_tensor("xin", [DIN, S], BF16, kind="ExternalInput")
    w1 = nc.dram_tensor("w1", [DIN, 12 * 128], BF16, kind="ExternalInput")
    if fp8_dr:
        # DoubleRow-packed W2: [128, (k_half 2), 1536] as [128, 3072] fp8
        w2q = nc.dram_tensor("w2q", [128, 2 * 1536], FP8, kind="ExternalInput")
        w2r = nc.dram_tensor("w2r", [128, 2 * 1536], FP8, kind="ExternalInput")
    else:
        w2a = nc.dram_tensor("w2a", [128, NNETS * 256], BF16, kind="ExternalInput")
        w2b = nc.dram_tensor("w2b", [128, NNETS * 256], BF16, kind="ExternalInput")
    b1 = nc.dram_tensor("b1", [128, 12], F32, kind="ExternalInput")
    b2 = nc.dram_tensor("b2", [128, 12], F32, kind="ExternalInput")
    outt = nc.dram_tensor("zsum", [128, 12 * slots], F32, kind="ExternalOutput")

    l2_scale = W2_SCALE if fp8_dr else 1.0
    # custom-op scalars, folded for psum values = l2_scale * a2
    dve_s0 = -GELU_GB * l2_scale
    dve_s1 = GELU_A2 / l2_scale
    dve_imm2 = GELU_B2 / (l2_scale * l2_scale * l2_scale)

    with tile.TileContext(nc) as tc:
        with (
            tc.tile_pool(name="wts", bufs=1) as wts,
            tc.tile_pool(name="xp", bufs=3) as xp,
            tc.tile_pool(name="h1p", bufs=2) as h1p,
            tc.tile_pool(name="h2p", bufs=2) as h2p,
            tc.tile_pool(name="scp", bufs=1) as scp,
            tc.tile_pool(name="op", bufs=1) as op,
            tc.tile_pool(name="psmA", bufs=1, space="PSUM") as psmA,
            tc.tile_pool(name="psmB", bufs=1, space="PSUM") as psmB,
        ):
            w1t = wts.tile([DIN, 12 * 128], BF16)
            nc.sync.dma_start(w1t, w1[:, :])
            if fp8_dr:
                w2qt = wts.tile([128, 2 * 1536], FP8)
                nc.sync.dma_start(w2qt, w2q[:, :])
                w2rt = wts.tile([128, 2 * 1536], FP8)
                nc.sync.dma_start(w2rt, w2r[:, :])
                w2q3 = w2qt.rearrange("p (two m) -> p two m", two=2)
                w2r3 = w2rt.rearrange("p (two m) -> p two m", two=2)
            else:
                w2t = [wts.tile([128, NNETS * 256], BF16, name=f"w2_{k}")
                       for k in range(2)]
                nc.sync.dma_start(w2t[0], w2a[:, :])
                nc.sync.dma_start(w2t[1], w2b[:, :])
            b1t = wts.tile([128, 12], F32)
            nc.sync.dma_start(b1t, b1[:, :])
            b2t = wts.tile([128, 12], F32)
            nc.sync.dma_start(b2t, b2[:, :])
            zsb = op.tile([128, 12 * slots], F32)
            zsb3 = zsb.rearrange("p (j s) -> p j s", j=12)

            def pieces_of(w):
                used = win_cols[w]
                return used, [
                    (p, min(p + PIECE, used)) for p in range(0, used, PIECE)
                ]

            # ---- software-pipelined window loop: window w's L1 interleaves
            # with window w-1's L2 on the PE so it never starves long enough
            # to drop out of the HAM warm clock (K=8/8, 2.4 GHz).
            state = {}

            def start_window(w):
                used, pieces = pieces_of(w)
                xw = xp.tile([DIN, WIN], BF16, tag="xw", name=f"xw{w}")
                nc.sync.dma_start(xw[:, :used], xin[:, w * WIN : w * WIN + used])
                h1a = h1p.tile([128, 12 * WIN], h1_dt, tag="h1", name=f"h1_{w}")
                h2a = h2p.tile([128, 12 * WIN], BF16, tag="h2", name=f"h2_{w}")
                # k-half pairs interleaved per column so DoubleRow's pair dim
                # is stride-1: net n, col c, half k at n*2*WIN + 2c + k
                h1v = h1a.rearrange("p (c two) -> p two c", two=2) if fp8_dr else None
                state[w] = (used, pieces, xw, h1a, h2a, h1v)

            def l1_tile(w, j):
                used, pieces, xw, h1a, h2a, h1v = state[w]
                n, k = j // 2, j % 2
                ps = psmA.tile([128, WIN], F32, tag="psA", name=f"ps1_{w}_{j}")
                for a, b in pieces:
                    nc.tensor.matmul(
                        ps[:, a:b],
                        w1t[:, j * 128 : (j + 1) * 128],
                        xw[:, a:b],
                        start=True,
                        stop=True,
                    )
                h1dst = (
                    h1v[:, k, n * WIN : n * WIN + used]
                    if fp8_dr
                    else h1a[:, j * WIN : j * WIN + used]
                )
                nc.scalar.activation(
                    h1dst, ps[:, :used], act, bias=b1t[:, j : j + 1],
                )

            def l2_tile(w, j):
                used, pieces, xw, h1a, h2a, h1v = state[w]
                n, mo = j // 2, j % 2
                ps = psmB.tile([128, WIN], F32, tag="psB", name=f"ps2_{w}_{j}")
                c0 = n * 256 + mo * 128
                if fp8_dr:
                    for a, b in pieces:
                        rhs = h1v[:, :, n * WIN + a : n * WIN + b]
                        nc.tensor.matmul(
                            ps[:, a:b],
                            w2q3[:, :, c0 : c0 + 128],
                            rhs,
                            start=True,
                            stop=False,
                            perf_mode=mybir.MatmulPerfMode.DoubleRow,
                            skip_group_check=True,
                        )
                        nc.tensor.matmul(
                            ps[:, a:b],
                            w2r3[:, :, c0 : c0 + 128],
                            rhs,
                            start=False,
                            stop=True,
                            perf_mode=mybir.MatmulPerfMode.DoubleRow,
                            skip_group_check=True,
                        )
                else:
                    for k in range(2):
                        for a, b in pieces:
                            nc.tensor.matmul(
                                ps[:, a:b],
                                w2t[k][:, c0 : c0 + 128],
                                h1a[:, (2 * n + k) * WIN + a
                                    : (2 * n + k) * WIN + b],
                                start=(k == 0),
                                stop=(k == 1),
                                skip_group_check=True,
                            )
                if j in dve_l2:
                    nc.vector._custom_dve(
                        gelu_op,
                        out=h2a[:, j * WIN : j * WIN + used],
                        in0=ps[:, :used],
                        s0=dve_s0, s1=dve_s1, imm2=dve_imm2,
                    )
                else:
                    nc.scalar.activation(
                        h2a[:, j * WIN : j * WIN + used], ps[:, :used],
                        act, bias=b2t[:, j : j + 1],
                        scale=1.0 / l2_scale,
                    )

            def pool_window(w):
                used, pieces, xw, h1a, h2a, h1v = state[w]
                h2a3 = h2a.rearrange("p (j c) -> p j c", j=12)
                sc = scp.tile([128, 12 * (WIN // 2)], BF16, tag="sc",
                              name=f"sc_{w}")
                sc3 = sc.rearrange("p (j c) -> p j c", j=12)
                sc2 = scp.tile([128, 12 * (WIN // 4)], BF16, tag="sc2",
                               name=f"sc2_{w}")
                sc23 = sc2.rearrange("p (j c) -> p j c", j=12)
                for s, off, L in slots_per_win[w]:
                    h = L // 2
                    q = L // 4
                    o2 = off // 2
                    o4 = off // 4
                    nc.vector.tensor_tensor(
                        sc3[:, :, o2 : o2 + h],
                        h2a3[:, :, off : off + h],
                        h2a3[:, :, off + h : off + L],
                        op=mybir.AluOpType.add,
                    )
                    nc.vector.tensor_tensor(
                        sc23[:, :, o4 : o4 + q],
                        sc3[:, :, o2 : o2 + q],
                        sc3[:, :, o2 + q : o2 + h],
                        op=mybir.AluOpType.add,
                    )
                    nc.vector.tensor_reduce(
                        zsb3[:, :, s : s + 1],
                        sc23[:, :, o4 : o4 + q],
                        axis=mybir.AxisListType.X,
                        op=mybir.AluOpType.add,
                    )
                del state[w]

            for w in range(nw + 1):
                if w < nw:
                    start_window(w)
                for j in range(12):
                    if w < nw:
                        l1_tile(w, j)
                    if w >= 1:
                        l2_tile(w - 1, j)
                if w >= 1:
                    pool_window(w - 1)
            nc.sync.dma_start(outt[:, :], zsb)
    nc.compile()
    return nc


# ----------------------------------------------------------------------------
# Host-side weight packing
# ----------------------------------------------------------------------------


def _collect(ins):
    W1s = [ins["router_W1"]] + [ins["e_W1"][i] for i in range(2)] + [
        ins["d_W1"][i] for i in range(3)
    ]
    W2s = [ins["router_W2"]] + [ins["e_W2"][i] for i in range(2)] + [
        ins["d_W2"][i] for i in range(3)
    ]
    Whs = [ins["router_Wh"]] + [ins["e_Wh"][i] for i in range(2)] + [
        ins["d_Wh"][i] for i in range(3)
    ]
    b1s = [ins["router_b1"]] + [ins["e_b1"][i] for i in range(2)] + [
        ins["d_b1"][i] for i in range(3)
    ]
    b2s = [ins["router_b2"]] + [ins["e_b2"][i] for i in range(2)] + [
        ins["d_b2"][i] for i in range(3)
    ]
    bhs = [ins["router_bh"]] + [ins["e_bh"][i] for i in range(2)] + [
        ins["d_bh"][i] for i in range(3)
    ]
    f = lambda a: np.ascontiguousarray(np.asarray(a, np.float32))
    return ([f(w) for w in W1s], [f(w) for w in W2s], [f(w) for w in Whs],
            [f(b) for b in b1s], [f(b) for b in b2s], [f(b) for b in bhs])


def pack_weights(ins, fp8_dr):
    W1s, W2s, Whs, b1s, b2s, bhs = _collect(ins)
    W1cat = np.concatenate(W1s, axis=1)  # [9, 1536]
    b1cat = np.concatenate(b1s)  # [1536]
    b2cat = np.concatenate(b2s)
    bhcat = np.concatenate(bhs)  # [19]
    b1t = b1cat.reshape(12, 128).T.copy()  # [128, 12]
    b2t = b2cat.reshape(12, 128).T.copy()
    bf = lambda a: a.astype(NPBF16)
    out = dict(
        w1=bf(W1cat), b1=b1t, b2=b2t, bhcat=bhcat, Whs=Whs,
        biases_zero=bool(
            np.all(b1cat == 0) and np.all(b2cat == 0)
        ),
    )
    # v3 bf16 packing (fallback path + non-DR)
    out["w2a"] = bf(np.concatenate([w[0:128, :] for w in W2s], axis=1))
    out["w2b"] = bf(np.concatenate([w[128:256, :] for w in W2s], axis=1))
    # pad-column contribution per h2 feature (exact; zero when biases zero)
    h1c = _gelu(b1cat)
    h2c_cat = np.zeros(1536, np.float32)
    for n in range(NNETS):
        a2c = h1c[n * 256 : (n + 1) * 256] @ W2s[n] + b2s[n]
        h2c_cat[n * 256 : (n + 1) * 256] = _gelu(a2c)
    out["h2c_cat"] = h2c_cat
    if fp8_dr:
        # DR layout: w2q[p, k, n*256+mo*128+m] = Q(16*W2_n)[k*128+p, mo*128+m]
        Wq = np.zeros((128, 2, 1536), np.float32)
        Wr = np.zeros((128, 2, 1536), np.float32)
        for n in range(NNETS):
            Wn = W2s[n] * np.float32(W2_SCALE)  # [256, 256]
            Q = Wn.astype(NPFP8).astype(np.float32)
            R = Wn - Q
            Rq = R.astype(NPFP8).astype(np.float32)
            for k in range(2):
                Wq[:, k, n * 256 : (n + 1) * 256] = Q[k * 128 : (k + 1) * 128, :]
                Wr[:, k, n * 256 : (n + 1) * 256] = Rq[k * 128 : (k + 1) * 128, :]
        out["w2q"] = Wq.reshape(128, 3072).astype(NPFP8)
        out["w2r"] = Wr.reshape(128, 3072).astype(NPFP8)
    return out


def build_xall(x, batch_ids, lay):
    """Scatter points into per-core feature-major padded streams [8, 9, S]."""
    counts = np.bincount(batch_ids, minlength=B)
    seg_start = np.zeros(B, np.int64)
    np.cumsum(counts[:-1], out=seg_start[1:])
    rank = np.empty(B, np.int64)
    rank[lay["ev"].reshape(-1)] = np.arange(B)
    r = rank[batch_ids]
    s = r // N_CORES
    c = r % N_CORES
    pos = np.arange(N_PTS) - seg_start[batch_ids]
    col = lay["slot_win"][s] * WIN + lay["slot_off"][s] + pos
    S = lay["nw"] * WIN
    xall = np.zeros((N_CORES, DIN, S), NPBF16)
    xall[c, :, col] = x.astype(NPBF16)
    return xall


# ----------------------------------------------------------------------------
# Host-side final mixing (exactly mirrors the reference)
# ----------------------------------------------------------------------------


def mix_outputs(y):
    """y: [B, 19] per-event head outputs -> [B, 11] model output."""
    y = y.astype(np.float32)
    morph = y[:, 0:6]
    m = morph - morph.max(axis=1, keepdims=True)
    e = np.exp(m)
    probs = e / e.sum(axis=1, keepdims=True)
    probs = np.maximum(probs, np.float32(1e-6))
    p_cont = probs[:, [0, 1]].sum(1, keepdims=True)
    p_uncont = probs[:, [2, 3, 5]].sum(1, keepdims=True)
    energy = p_cont * y[:, 6:8] + p_uncont * y[:, 8:10]
    p_cas = probs[:, 0:1]
    p_track = probs[:, [1, 2, 3, 5]].sum(1, keepdims=True)
    gate = 1.0 / (1.0 + np.exp(-(energy[:, 0:1] - np.float32(4.0))))
    dirp = p_cas * y[:, 10:13] + p_track * (
        (1.0 - gate) * y[:, 13:16] + gate * y[:, 16:19]
    )
    return np.concatenate([morph, energy, dirp], axis=1).astype(np.float32)


def postprocess(zsums, lay, wp, counts, dve_l2, fp8_dr):
    """zsums: [8][128, 12*SLOTS] pooled-h2 sums -> [B, 11]."""
    y = np.zeros((B, ZD), np.float32)
    ev = lay["ev"]
    slot_len = lay["slot_len"]
    h2c = wp["h2c_cat"]
    # per-tile output scale: DVE tiles produce 2*l2_scale*gelu
    tile_scale = np.ones(12, np.float32)
    for j in dve_l2:
        tile_scale[j] = 1.0 / (2.0 * (W2_SCALE if fp8_dr else 1.0))
    for c in range(N_CORES):
        zf = zsums[c]  # [128, 12*SLOTS]; col j*SLOTS+s = features of tile j
        pooled = (
            zf.reshape(128, 12, SLOTS) * tile_scale[None, :, None]
        ).transpose(2, 1, 0).reshape(SLOTS, 1536)
        e = ev[:, c]
        cnt = counts[e].astype(np.float32)
        pad = (slot_len - counts[e]).astype(np.float32)
        pooled = (pooled - pad[:, None] * h2c[None, :]) / np.maximum(cnt, 1.0)[
            :, None
        ]
        yy = np.zeros((SLOTS, ZD), np.float32)
        for n in range(NNETS):
            yy[:, ZOFF[n] : ZOFF[n] + ZDIMS[n]] = (
                pooled[:, n * 256 : (n + 1) * 256] @ wp["Whs"][n]
            )
        y[e] = yy + wp["bhcat"][None, :]
    return mix_outputs(y)


# ----------------------------------------------------------------------------
# Entry point
# ----------------------------------------------------------------------------

_CACHE = {}
_LAST_RESULT = None  # set when KERNEL_TRACE=1; holds BassKernelResults


def kernel(**inputs):
    import os

    global _LAST_RESULT
    from concourse.bass_utils import run_bass_kernel_spmd

    ins = {k: np.asarray(v) for k, v in inputs.items()}
    coords = ins["coords"].astype(np.float32)
    features = ins["features"].astype(np.float32)
    batch_ids = ins["batch_ids"].astype(np.int64)
    x = np.concatenate([coords, features], axis=1)  # [N, 9]

    counts = np.bincount(batch_ids, minlength=B)
    lay = build_layout(counts)
    fp8_dr = FP8_DR
    dve_l2 = DVE_L2
    wp = pack_weights(ins, fp8_dr)
    if not wp["biases_zero"]:
        # approximation paths assume zero biases (as generated by
        # setup_inputs); fall back to the exact all-ACT bf16 program
        fp8_dr = False
        dve_l2 = ()
    xall = build_xall(x, batch_ids, lay)

    key = (lay["nw"], tuple(map(tuple, (tuple(w) for w in lay["slots_per_win"]))))
    key = (key, tuple(lay["win_cols"]), dve_l2, fp8_dr)
    if key not in _CACHE:
        _CACHE[key] = build_program(
            lay["nw"], lay["slots_per_win"], win_cols=lay["win_cols"],
            dve_l2=dve_l2, fp8_dr=fp8_dr,
        )
    nc = _CACHE[key]

    shared = {k: wp[k] for k in ("w1", "b1", "b2")}
    if fp8_dr:
        shared["w2q"] = wp["w2q"]
        shared["w2r"] = wp["w2r"]
    else:
        shared["w2a"] = wp["w2a"]
        shared["w2b"] = wp["w2b"]
    in_maps = [dict(shared, xin=np.ascontiguousarray(xall[c]))
               for c in range(N_CORES)]
    trace = bool(int(os.environ.get("KERNEL_TRACE", "0")))
    res = run_bass_kernel_spmd(
        nc, in_maps, core_ids=list(range(N_CORES)), trace=trace
    )
    _LAST_RESULT = res
    zsums = [res.results[c]["zsum"] for c in range(N_CORES)]
    return postprocess(zsums, lay, wp, counts, dve_l2, fp8_dr)


# revision 11
# speedup vs baseline: 1.0338x; 1.0338x over previous
"""Trainium2 Bass kernel for nn_NeptuneMoEModel (moe_routing).

Model: 6 small MLPs (router + 2 energy experts + 3 direction experts) over
N=262144 points -> segment-mean-pool into B=1024 events -> tiny per-event
head/mixing math.

v4 strategy (8 NeuronCores, SPMD, data-parallel over events):
  - Events sorted by point count, round-robin assigned to cores; slots
    first-fit packed into 2048-column windows (as v3 baseline).
  - L1 = fused [9 -> 1536] bf16 matmul; gelu on the scalar engine (ACT)
    writing h1 as fp8e4 into one combined [128, 12*WIN] tile.
  - L2 = fp8 DoubleRow matmuls (K=256 in one pass, 2 moving rows/cycle):
    W2 quantized to e4m3 at scale 16 plus an e4m3 residual-correction
    matmul, accumulating in fp32 PSUM.  ~1.7x faster than bf16 on the PE.
  - L2 gelu split between ACT (scale=1/16 + bias) and a custom DVE
    micro-op (8-ALU-stage clamped cubic sigmoid with exact tails; computes
    32*gelu(psum/16); host divides those pooled sums by 32).  This offloads
    the ACT bottleneck onto the otherwise underused vector engine.
  - Pooling: h2 lives in one combined [128, 12*WIN] tile; per slot one
    bf16 tensor_tensor halving pass + one [12, L/2] tensor_reduce
    (~3x fewer DVE cycles than v3's 12 separate reduces per slot).
  - Host: divide by counts, per-tile scales, head matmuls, softmax/gating
    mixing - all O(B*1536) numpy.

Falls back to an all-bf16/all-ACT program (v3 baseline behavior) when
biases are nonzero; setup_inputs uses zero biases so the harness always
takes the fast path.
"""

import sys

sys.path.insert(0, "/opt/trn_rl_repo")

import numpy as np

import concourse.bass as bass
import concourse.mybir as mybir
import concourse.tile as tile
from concourse import bacc

N_CORES = 8
B = 1024
N_PTS = 262144
DIN = 9
H = 256
NNETS = 6
ZDIMS = [6, 2, 2, 3, 3, 3]
ZOFF = [0, 6, 8, 10, 13, 16]
ZD = 19
WIN = 2048
PIECE = 512
SLOTS = B // N_CORES  # 128
F32 = mybir.dt.float32
BF16 = mybir.dt.bfloat16
FP8 = mybir.dt.float8e4
try:
    import ml_dtypes

    NPBF16 = ml_dtypes.bfloat16
    NPFP8 = ml_dtypes.float8_e4m3
except ImportError:  # pragma: no cover
    NPBF16 = None
    NPFP8 = None
GELU = mybir.ActivationFunctionType.Gelu_apprx_tanh

# ---- custom DVE gelu: out = x * (1 + u*(A2 + B2*u^2)), u = clamp(x, -GB, GB)
# == 2*gelu_approx(x); GB*(A2 + B2*GB^2) = 2*0.5 = 1 makes the tails exact
# (out = 2x for x > GB, out = 0 for x < -GB).
GELU_GB = 2.05995
_GA = 0.364086
_GBB = (0.5 / GELU_GB - _GA) / GELU_GB**2
GELU_A2 = 2 * _GA
GELU_B2 = 2 * _GBB

# which L2 j-tiles (j = 2*net + mo) run their gelu on the DVE custom op
DVE_L2 = (4, 5, 8, 9, 10, 11)
FP8_DR = True  # L2 via fp8 DoubleRow + residual
W2_SCALE = 16.0  # e4m3 quantization scale for W2 (and its residual)

_GELU_OP = None


def _register_gelu_op():
    """Register the custom DVE op at runtime (no repo edits needed)."""
    global _GELU_OP
    if _GELU_OP is not None:
        return _GELU_OP
    from concourse import dve_ops as dvo
    from concourse.dve_spec import Spec, Src0, C0, C1, C2, Zero, One, lower
    from concourse.dve_spec import maxx, minn, sq
    from concourse.dve_uop import DveOpSpec

    name = "GELU2X_CCS_ANT"
    for op in dvo.OPS:
        if op.name == name:
            _GELU_OP = op
            return op

    u = minn(maxx(Src0, C0), Zero - C0)
    body = ((sq(u) * C2 + C1) * u + One) * Src0

    def ref(in0, s0, s1, imm2):
        uu = np.clip(in0, s0, -s0)
        return ((uu * uu * imm2 + s1) * uu + 1.0) * in0

    spec = Spec(body=body, reference=ref)
    row = dvo._CUSTOM_DVE_ROW_BASE + len(dvo.OPS)
    assert row < 0x20, "custom DVE opcode rows exhausted"
    dvo._SUB_OPCODE_FOR_NAME[name] = row
    shas = {}
    for ver in ("v3", "v4"):
        o = DveOpSpec(name=name, opcode=row, uops=lower(spec, ver=ver), rd1_en=False)
        shas[ver] = o.sha(ver)
    op = dvo.DveOp(name=name, spec=spec, subdim=False, uops_sha=shas)
    dvo.OPS.append(op)
    dvo.CUSTOM_DVE_SPECS[name] = spec
    _GELU_OP = op
    return op


def _gelu(x):
    """jax.nn.gelu(approximate=True) in numpy/fp32."""
    x = np.asarray(x, np.float32)
    c = np.float32(np.sqrt(2.0 / np.pi))
    return (0.5 * x * (1.0 + np.tanh(c * (x + 0.044715 * x * x * x)))).astype(
        np.float32
    )


# ----------------------------------------------------------------------------
# Layout: event -> (core, slot); slots -> windows
# ----------------------------------------------------------------------------


def build_layout(counts):
    counts = np.asarray(counts)
    order = np.argsort(-counts, kind="stable")
    ev = order.reshape(SLOTS, N_CORES)  # ev[s, c] = event id
    slot_len = counts[ev].max(1)
    # multiples of 8 keep both halving passes' operand offsets even (4B
    # aligned) so the DVE 2x_1p packed mode stays engaged
    slot_len = np.maximum(((slot_len + 7) // 8) * 8, 8).astype(np.int64)
    assert slot_len.max() <= WIN
    # first-fit (slot_len is non-increasing -> this is first-fit-decreasing)
    win_used = []
    slot_win = np.zeros(SLOTS, np.int64)
    slot_off = np.zeros(SLOTS, np.int64)
    for s in range(SLOTS):
        L = int(slot_len[s])
        for w in range(len(win_used)):
            if win_used[w] + L <= WIN:
                slot_win[s] = w
                slot_off[s] = win_used[w]
                win_used[w] += L
                break
        else:
            slot_win[s] = len(win_used)
            slot_off[s] = 0
            win_used.append(L)
    nw = len(win_used)
    slots_per_win = [[] for _ in range(nw)]
    for s in range(SLOTS):
        slots_per_win[slot_win[s]].append(
            (s, int(slot_off[s]), int(slot_len[s]))
        )
    win_cols = [min(WIN, ((u + 7) // 8) * 8) for u in win_used]
    return dict(
        ev=ev,
        slot_len=slot_len,
        slot_win=slot_win,
        slot_off=slot_off,
        nw=nw,
        slots_per_win=slots_per_win,
        win_cols=win_cols,
    )


# ----------------------------------------------------------------------------
# Device program (v4)
# ----------------------------------------------------------------------------


def build_program(nw, slots_per_win, win_cols=None, slots=SLOTS, act=GELU,
                  dve_l2=DVE_L2, fp8_dr=FP8_DR):
    gelu_op = _register_gelu_op()
    nc = bacc.Bacc(None, target_bir_lowering=False)
    if win_cols is None:
        win_cols = [WIN] * nw
    S = nw * WIN
    h1_dt = FP8 if fp8_dr else BF16
    xin = nc.dram_tensor("xin", [DIN, S], BF16, kind="ExternalInput")
    w1 = nc.dram_tensor("w1", [DIN, 12 * 128], BF16, kind="ExternalInput")
    if fp8_dr:
        # DoubleRow-packed W2: [128, (k_half 2), 1536] as [128, 3072] fp8
        w2q = nc.dram_tensor("w2q", [128, 2 * 1536], FP8, kind="ExternalInput")
        w2r = nc.dram_tensor("w2r", [128, 2 * 1536], FP8, kind="ExternalInput")
    else:
        w2a = nc.dram_tensor("w2a", [128, NNETS * 256], BF16, kind="ExternalInput")
        w2b = nc.dram_tensor("w2b", [128, NNETS * 256], BF16, kind="ExternalInput")
    b1 = nc.dram_tensor("b1", [128, 12], F32, kind="ExternalInput")
    b2 = nc.dram_tensor("b2", [128, 12], F32, kind="ExternalInput")
    outt = nc.dram_tensor("zsum", [128, 12 * slots], F32, kind="ExternalOutput")

    l2_scale = W2_SCALE if fp8_dr else 1.0
    # custom-op scalars, folded for psum values = l2_scale * a2
    dve_s0 = -GELU_GB * l2_scale
    dve_s1 = GELU_A2 / l2_scale
    dve_imm2 = GELU_B2 / (l2_scale * l2_scale * l2_scale)

    with tile.TileContext(nc) as tc:
        with (
            tc.tile_pool(name="wts", bufs=1) as wts,
            tc.tile_pool(name="xp", bufs=3) as xp,
            tc.tile_pool(name="h1p", bufs=2) as h1p,
            tc.tile_pool(name="h2p", bufs=2) as h2p,
            tc.tile_pool(name="scp", bufs=1) as scp,
            tc.tile_pool(name="op", bufs=1) as op,
            tc.tile_pool(name="psmA", bufs=1, space="PSUM") as psmA,
            tc.tile_pool(name="psmB", bufs=1, space="PSUM") as psmB,
        ):
            w1t = wts.tile([DIN, 12 * 128], BF16)
            nc.sync.dma_start(w1t, w1[:, :])
            if fp8_dr:
                w2qt = wts.tile([128, 2 * 1536], FP8)
                nc.sync.dma_start(w2qt, w2q[:, :])
                w2rt = wts.tile([128, 2 * 1536], FP8)
                nc.sync.dma_start(w2rt, w2r[:, :])
                w2q3 = w2qt.rearrange("p (two m) -> p two m", two=2)
                w2r3 = w2rt.rearrange("p (two m) -> p two m", two=2)
            else:
                w2t = [wts.tile([128, NNETS * 256], BF16, name=f"w2_{k}")
                       for k in range(2)]
                nc.sync.dma_start(w2t[0], w2a[:, :])
                nc.sync.dma_start(w2t[1], w2b[:, :])
            b1t = wts.tile([128, 12], F32)
            nc.sync.dma_start(b1t, b1[:, :])
            b2t = wts.tile([128, 12], F32)
            nc.sync.dma_start(b2t, b2[:, :])
            zsb = op.tile([128, 12 * slots], F32)
            zsb3 = zsb.rearrange("p (j s) -> p j s", j=12)

            def pieces_of(w):
                used = win_cols[w]
                return used, [
                    (p, min(p + PIECE, used)) for p in range(0, used, PIECE)
                ]

            # ---- software-pipelined window loop: window w's L1 interleaves
            # with window w-1's L2 on the PE so it never starves long enough
            # to drop out of the HAM warm clock (K=8/8, 2.4 GHz).
            state = {}

            def start_window(w):
                used, pieces = pieces_of(w)
                xw = xp.tile([DIN, WIN], BF16, tag="xw", name=f"xw{w}")
                nc.sync.dma_start(xw[:, :used], xin[:, w * WIN : w * WIN + used])
                h1a = h1p.tile([128, 12 * WIN], h1_dt, tag="h1", name=f"h1_{w}")
                h2a = h2p.tile([128, 12 * WIN], BF16, tag="h2", name=f"h2_{w}")
                # k-half pairs interleaved per column so DoubleRow's pair dim
                # is stride-1: net n, col c, half k at n*2*WIN + 2c + k
                h1v = h1a.rearrange("p (c two) -> p two c", two=2) if fp8_dr else None
                state[w] = (used, pieces, xw, h1a, h2a, h1v)

            def l1_tile(w, j):
                used, pieces, xw, h1a, h2a, h1v = state[w]
                n, k = j // 2, j % 2
                ps = psmA.tile([128, WIN], F32, tag="psA", name=f"ps1_{w}_{j}")
                for a, b in pieces:
                    nc.tensor.matmul(
                        ps[:, a:b],
                        w1t[:, j * 128 : (j + 1) * 128],
                        xw[:, a:b],
                        start=True,
                        stop=True,
                    )
                h1dst = (
                    h1v[:, k, n * WIN : n * WIN + used]
                    if fp8_dr
                    else h1a[:, j * WIN : j * WIN + used]
                )
                nc.scalar.activation(
                    h1dst, ps[:, :used], act, bias=b1t[:, j : j + 1],
                )

            def l2_tile(w, j):
                used, pieces, xw, h1a, h2a, h1v = state[w]
                n, mo = j // 2, j % 2
                ps = psmB.tile([128, WIN], F32, tag="psB", name=f"ps2_{w}_{j}")
                c0 = n * 256 + mo * 128
                if fp8_dr:
                    for a, b in pieces:
                        rhs = h1v[:, :, n * WIN + a : n * WIN + b]
                        nc.tensor.matmul(
                            ps[:, a:b],
                            w2q3[:, :, c0 : c0 + 128],
                            rhs,
                            start=True,
                            stop=False,
                            perf_mode=mybir.MatmulPerfMode.DoubleRow,
                            skip_group_check=True,
                        )
                        nc.tensor.matmul(
                            ps[:, a:b],
                            w2r3[:, :, c0 : c0 + 128],
                            rhs,
                            start=False,
                            stop=True,
                            perf_mode=mybir.MatmulPerfMode.DoubleRow,
                            skip_group_check=True,
                        )
                else:
                    for k in range(2):
                        for a, b in pieces:
                            nc.tensor.matmul(
                                ps[:, a:b],
                                w2t[k][:, c0 : c0 + 128],
                                h1a[:, (2 * n + k) * WIN + a
                                    : (2 * n + k) * WIN + b],
                                start=(k == 0),
                                stop=(k == 1),
                                skip_group_check=True,
                            )
                if j in dve_l2:
                    nc.vector._custom_dve(
                        gelu_op,
                        out=h2a[:, j * WIN : j * WIN + used],
                        in0=ps[:, :used],
                        s0=dve_s0, s1=dve_s1, imm2=dve_imm2,
                    )
                else:
                    nc.scalar.activation(
                        h2a[:, j * WIN : j * WIN + used], ps[:, :used],
                        act, bias=b2t[:, j : j + 1],
                        scale=1.0 / l2_scale,
                    )

            pool_sc = {}

            def pool_slot(w, slot):
                used, pieces, xw, h1a, h2a, h1v = state[w]
                h2a3 = h2a.rearrange("p (j c) -> p j c", j=12)
                if w not in pool_sc:
                    sc = scp.tile([128, 12 * (WIN // 2)], BF16, tag="sc",
                                  name=f"sc_{w}")
                    sc2 = scp.tile([128, 12 * (WIN // 4)], BF16, tag="sc2",
                                   name=f"sc2_{w}")
                    pool_sc[w] = (
                        sc.rearrange("p (j c) -> p j c", j=12),
                        sc2.rearrange("p (j c) -> p j c", j=12),
                    )
                sc3, sc23 = pool_sc[w]
                s, off, L = slot
                h = L // 2
                q = L // 4
                o2 = off // 2
                o4 = off // 4
                nc.vector.tensor_tensor(
                    sc3[:, :, o2 : o2 + h],
                    h2a3[:, :, off : off + h],
                    h2a3[:, :, off + h : off + L],
                    op=mybir.AluOpType.add,
                )
                nc.vector.tensor_tensor(
                    sc23[:, :, o4 : o4 + q],
                    sc3[:, :, o2 : o2 + q],
                    sc3[:, :, o2 + q : o2 + h],
                    op=mybir.AluOpType.add,
                )
                nc.vector.tensor_reduce(
                    zsb3[:, :, s : s + 1],
                    sc23[:, :, o4 : o4 + q],
                    axis=mybir.AxisListType.X,
                    op=mybir.AluOpType.add,
                )

            # L2 emission order: ACT-gelu'd tiles first, DVE-gelu'd last, so
            # the PE's first six L2 psum recycles never wait on the DVE (which
            # drains window w-2's pooling early in the iteration).
            j_seq = [j for j in range(12) if j not in dve_l2] + list(dve_l2)

            for w in range(nw + 2):
                if w < nw:
                    start_window(w)
                pw = w - 2
                pool_list = slots_per_win[pw] if 0 <= pw < nw else []
                pi = 0
                for idx in range(12):
                    if w < nw:
                        l1_tile(w, idx)
                    if 1 <= w <= nw:
                        l2_tile(w - 1, j_seq[idx])
                    # spread pool(w-2) slots across the iteration
                    want = ((idx + 1) * len(pool_list)) // 12
                    while pi < want:
                        pool_slot(pw, pool_list[pi])
                        pi += 1
                if pool_list:
                    while pi < len(pool_list):
                        pool_slot(pw, pool_list[pi])
                        pi += 1
                    pool_sc.pop(pw, None)
                    del state[pw]
            nc.sync.dma_start(outt[:, :], zsb)
    nc.compile()
    return nc


# ----------------------------------------------------------------------------
# Host-side weight packing
# ----------------------------------------------------------------------------


def _collect(ins):
    W1s = [ins["router_W1"]] + [ins["e_W1"][i] for i in range(2)] + [
        ins["d_W1"][i] for i in range(3)
    ]
    W2s = [ins["router_W2"]] + [ins["e_W2"][i] for i in range(2)] + [
        ins["d_W2"][i] for i in range(3)
    ]
    Whs = [ins["router_Wh"]] + [ins["e_Wh"][i] for i in range(2)] + [
        ins["d_Wh"][i] for i in range(3)
    ]
    b1s = [ins["router_b1"]] + [ins["e_b1"][i] for i in range(2)] + [
        ins["d_b1"][i] for i in range(3)
    ]
    b2s = [ins["router_b2"]] + [ins["e_b2"][i] for i in range(2)] + [
        ins["d_b2"][i] for i in range(3)
    ]
    bhs = [ins["router_bh"]] + [ins["e_bh"][i] for i in range(2)] + [
        ins["d_bh"][i] for i in range(3)
    ]
    f = lambda a: np.ascontiguousarray(np.asarray(a, np.float32))
    return ([f(w) for w in W1s], [f(w) for w in W2s], [f(w) for w in Whs],
            [f(b) for b in b1s], [f(b) for b in b2s], [f(b) for b in bhs])


def pack_weights(ins, fp8_dr):
    W1s, W2s, Whs, b1s, b2s, bhs = _collect(ins)
    W1cat = np.concatenate(W1s, axis=1)  # [9, 1536]
    b1cat = np.concatenate(b1s)  # [1536]
    b2cat = np.concatenate(b2s)
    bhcat = np.concatenate(bhs)  # [19]
    b1t = b1cat.reshape(12, 128).T.copy()  # [128, 12]
    b2t = b2cat.reshape(12, 128).T.copy()
    bf = lambda a: a.astype(NPBF16)
    out = dict(
        w1=bf(W1cat), b1=b1t, b2=b2t, bhcat=bhcat, Whs=Whs,
        biases_zero=bool(
            np.all(b1cat == 0) and np.all(b2cat == 0)
        ),
    )
    # v3 bf16 packing (fallback path + non-DR)
    out["w2a"] = bf(np.concatenate([w[0:128, :] for w in W2s], axis=1))
    out["w2b"] = bf(np.concatenate([w[128:256, :] for w in W2s], axis=1))
    # pad-column contribution per h2 feature (exact; zero when biases zero)
    h1c = _gelu(b1cat)
    h2c_cat = np.zeros(1536, np.float32)
    for n in range(NNETS):
        a2c = h1c[n * 256 : (n + 1) * 256] @ W2s[n] + b2s[n]
        h2c_cat[n * 256 : (n + 1) * 256] = _gelu(a2c)
    out["h2c_cat"] = h2c_cat
    if fp8_dr:
        # DR layout: w2q[p, k, n*256+mo*128+m] = Q(16*W2_n)[k*128+p, mo*128+m]
        Wq = np.zeros((128, 2, 1536), np.float32)
        Wr = np.zeros((128, 2, 1536), np.float32)
        for n in range(NNETS):
            Wn = W2s[n] * np.float32(W2_SCALE)  # [256, 256]
            Q = Wn.astype(NPFP8).astype(np.float32)
            R = Wn - Q
            Rq = R.astype(NPFP8).astype(np.float32)
            for k in range(2):
                Wq[:, k, n * 256 : (n + 1) * 256] = Q[k * 128 : (k + 1) * 128, :]
                Wr[:, k, n * 256 : (n + 1) * 256] = Rq[k * 128 : (k + 1) * 128, :]
        out["w2q"] = Wq.reshape(128, 3072).astype(NPFP8)
        out["w2r"] = Wr.reshape(128, 3072).astype(NPFP8)
    return out


def build_xall(x, batch_ids, lay):
    """Scatter points into per-core feature-major padded streams [8, 9, S]."""
    counts = np.bincount(batch_ids, minlength=B)
    seg_start = np.zeros(B, np.int64)
    np.cumsum(counts[:-1], out=seg_start[1:])
    rank = np.empty(B, np.int64)
    rank[lay["ev"].reshape(-1)] = np.arange(B)
    r = rank[batch_ids]
    s = r // N_CORES
    c = r % N_CORES
    pos = np.arange(N_PTS) - seg_start[batch_ids]
    col = lay["slot_win"][s] * WIN + lay["slot_off"][s] + pos
    S = lay["nw"] * WIN
    xall = np.zeros((N_CORES, DIN, S), NPBF16)
    xall[c, :, col] = x.astype(NPBF16)
    return xall


# ----------------------------------------------------------------------------
# Host-side final mixing (exactly mirrors the reference)
# ----------------------------------------------------------------------------


def mix_outputs(y):
    """y: [B, 19] per-event head outputs -> [B, 11] model output."""
    y = y.astype(np.float32)
    morph = y[:, 0:6]
    m = morph - morph.max(axis=1, keepdims=True)
    e = np.exp(m)
    probs = e / e.sum(axis=1, keepdims=True)
    probs = np.maximum(probs, np.float32(1e-6))
    p_cont = probs[:, [0, 1]].sum(1, keepdims=True)
    p_uncont = probs[:, [2, 3, 5]].sum(1, keepdims=True)
    energy = p_cont * y[:, 6:8] + p_uncont * y[:, 8:10]
    p_cas = probs[:, 0:1]
    p_track = probs[:, [1, 2, 3, 5]].sum(1, keepdims=True)
    gate = 1.0 / (1.0 + np.exp(-(energy[:, 0:1] - np.float32(4.0))))
    dirp = p_cas * y[:, 10:13] + p_track * (
        (1.0 - gate) * y[:, 13:16] + gate * y[:, 16:19]
    )
    return np.concatenate([morph, energy, dirp], axis=1).astype(np.float32)


def postprocess(zsums, lay, wp, counts, dve_l2, fp8_dr):
    """zsums: [8][128, 12*SLOTS] pooled-h2 sums -> [B, 11]."""
    y = np.zeros((B, ZD), np.float32)
    ev = lay["ev"]
    slot_len = lay["slot_len"]
    h2c = wp["h2c_cat"]
    # per-tile output scale: DVE tiles produce 2*l2_scale*gelu
    tile_scale = np.ones(12, np.float32)
    for j in dve_l2:
        tile_scale[j] = 1.0 / (2.0 * (W2_SCALE if fp8_dr else 1.0))
    for c in range(N_CORES):
        zf = zsums[c]  # [128, 12*SLOTS]; col j*SLOTS+s = features of tile j
        pooled = (
            zf.reshape(128, 12, SLOTS) * tile_scale[None, :, None]
        ).transpose(2, 1, 0).reshape(SLOTS, 1536)
        e = ev[:, c]
        cnt = counts[e].astype(np.float32)
        pad = (slot_len - counts[e]).astype(np.float32)
        pooled = (pooled - pad[:, None] * h2c[None, :]) / np.maximum(cnt, 1.0)[
            :, None
        ]
        yy = np.zeros((SLOTS, ZD), np.float32)
        for n in range(NNETS):
            yy[:, ZOFF[n] : ZOFF[n] + ZDIMS[n]] = (
                pooled[:, n * 256 : (n + 1) * 256] @ wp["Whs"][n]
            )
        y[e] = yy + wp["bhcat"][None, :]
    return mix_outputs(y)


# ----------------------------------------------------------------------------
# Entry point
# ----------------------------------------------------------------------------

_CACHE = {}
_LAST_RESULT = None  # set when KERNEL_TRACE=1; holds BassKernelResults


def kernel(**inputs):
    import os

    global _LAST_RESULT
    from concourse.bass_utils import run_bass_kernel_spmd

    ins = {k: np.asarray(v) for k, v in inputs.items()}
    coords = ins["coords"].astype(np.float32)
    features = ins["features"].astype(np.float32)
    batch_ids = ins["batch_ids"].astype(np.int64)
    x = np.concatenate([coords, features], axis=1)  # [N, 9]

    counts = np.bincount(batch_ids, minlength=B)
    lay = build_layout(counts)
    fp8_dr = FP8_DR
    dve_l2 = DVE_L2
    wp = pack_weights(ins, fp8_dr)
    if not wp["biases_zero"]:
        # approximation paths assume zero biases (as generated by
        # setup_inputs); fall back to the exact all-ACT bf16 program
        fp8_dr = False
        dve_l2 = ()
    xall = build_xall(x, batch_ids, lay)

    key = (lay["nw"], tuple(map(tuple, (tuple(w) for w in lay["slots_per_win"]))))
    key = (key, tuple(lay["win_cols"]), dve_l2, fp8_dr)
    if key not in _CACHE:
        _CACHE[key] = build_program(
            lay["nw"], lay["slots_per_win"], win_cols=lay["win_cols"],
            dve_l2=dve_l2, fp8_dr=fp8_dr,
        )
    nc = _CACHE[key]

    shared = {k: wp[k] for k in ("w1", "b1", "b2")}
    if fp8_dr:
        shared["w2q"] = wp["w2q"]
        shared["w2r"] = wp["w2r"]
    else:
        shared["w2a"] = wp["w2a"]
        shared["w2b"] = wp["w2b"]
    in_maps = [dict(shared, xin=np.ascontiguousarray(xall[c]))
               for c in range(N_CORES)]
    trace = bool(int(os.environ.get("KERNEL_TRACE", "0")))
    res = run_bass_kernel_spmd(
        nc, in_maps, core_ids=list(range(N_CORES)), trace=trace
    )
    _LAST_RESULT = res
    zsums = [res.results[c]["zsum"] for c in range(N_CORES)]
    return postprocess(zsums, lay, wp, counts, dve_l2, fp8_dr)
